# revision 8
# baseline (speedup 1.0000x reference)
"""ConvTransE decoder scoring kernel for Trainium2 (8 NeuronCores, Bass/Tile).

Math (per batch row b):
    subj = entity_emb[triples[b,0]]; rel = rel_emb[triples[b,1]]
    obj  = entity_emb[triples[b,2]]
    combined = concat(subj, rel)                      # (1024,)
    conv[f,i] = relu(sum_k combined[i+k] * w[f,k] + cb[f])   # i in [0,1022)
    proj = conv.reshape(-1) @ fc + fc_bias            # (512,)
    score = proj . obj

Strategy: data-parallel over the batch (1024 triples per core).  Per core:
  - indirect-DMA gathers of subj/rel/obj rows
  - PE transposes build "combined^T" window tiles (conv positions on the
    partition axis, batch on the free axis)
  - conv as banded fp16 matmuls (host-precomputed band matrices)
  - relu on the scalar engine during PSUM->SBUF copy (conv bias folded in)
  - the 32704x512 FC contraction as fp16 matmuls accumulating in PSUM,
    streaming host-repacked fp16 fc chunks from DRAM
  - fc_bias folded in as one extra contraction row (paired with a 1.0 row)
  - final dot with obj rows on the vector engine
Batch is processed in two halves of 512 so PSUM holds 4 accumulator banks
plus double-buffered conv banks.

All matmul operand APs are kept at partition base 0 (nonzero-base operands
produce broken NEFFs on this toolchain).
"""
import numpy as np

import concourse.bass as bass
import concourse.mybir as mybir
import concourse.tile as tile
from concourse import bacc
from concourse.masks import make_identity
from concourse.bass_utils import run_bass_kernel_spmd

# problem constants
NE, NRE, D, F, KS, B = 100000, 500, 512, 32, 3, 8192
NCORES = 8
CONV_LEN = 2 * D - KS + 1          # 1022
W = 114                            # conv output rows per window
NW = 9                             # number of windows (9*114 >= 1022)
M_C = [W] * (NW - 1) + [CONV_LEN - W * (NW - 1)]       # [114]*8 + [110]
K_C = [m + KS - 1 for m in M_C]                        # [116]*8 + [112]
BAND_STRIDE = W                    # free-dim stride of per-filter band slabs
FDT = mybir.dt.float16

# transpose source pieces per window: (source, d0, length, dest)
# source: 0=subj, 1=rel;  dest: window index, or "4a"/"4b" for the split window
_PIECES = []
for _c in range(NW):
    p0, p1 = W * _c, W * _c + K_C[_c]
    if p1 <= D:
        _PIECES.append((0, p0, p1 - p0, _c))
    elif p0 >= D:
        _PIECES.append((1, p0 - D, p1 - p0, _c))
    else:
        _PIECES.append((0, p0, D - p0, "4a"))
        _PIECES.append((1, 0, p1 - D, "4b"))
_SPLIT_C = 4
_SPLIT_A = D - W * _SPLIT_C        # 56 rows from subj
_SPLIT_B = K_C[_SPLIT_C] - _SPLIT_A  # 60 rows from rel


def host_pack(conv_weight, conv_bias, fc, fc_bias):
    """Precompute fp16 band matrices and the repacked fp16 fc chunks."""
    w = np.asarray(conv_weight, np.float32).reshape(F, KS)
    # band slabs: bands[c][p, f*W + i] = w[f, p - i]; slab 9 = rows 56.. of c=4
    bands = np.zeros((NW + 1, 128, F * W), np.float32)
    for c in range(NW):
        m, k = M_C[c], K_C[c]
        sl = np.zeros((k, W))
        for kk in range(KS):
            idx = np.arange(m)
            sl[idx + kk, idx] = 1.0
        for f in range(F):
            bf = np.zeros((k, W))
            for kk in range(KS):
                idx = np.arange(m)
                bf[idx + kk, idx] = w[f, kk]
            if c == _SPLIT_C:
                bands[c, :_SPLIT_A, f * W:f * W + W] = bf[:_SPLIT_A]
                bands[NW, :_SPLIT_B, f * W:f * W + W] = bf[_SPLIT_A:]
            else:
                bands[c, :k, f * W:f * W + W] = bf
    bands16 = bands.astype(np.float16)

    fc32 = np.asarray(fc, np.float32)
    fcp = np.zeros((NW, 128, F, D), np.float32)
    for c in range(NW):
        m = M_C[c]
        rows = np.arange(m)
        for f in range(F):
            fcp[c, :m, f, :] = fc32[f * CONV_LEN + W * c + rows, :]
    # fc_bias as one extra contraction row on the last chunk (paired with a
    # 1.0 row in the relu output)
    fcp[NW - 1, M_C[NW - 1], F - 1, :] = np.asarray(fc_bias, np.float32)
    fcp16 = fcp.astype(np.float16)

    cbias = np.tile(np.asarray(conv_bias, np.float32).reshape(1, F), (128, 1))
    return bands16, fcp16, cbias.astype(np.float32)


def build_bass(bloc):
    """Build the per-core Bass module for a local batch of `bloc` triples."""
    assert bloc % 256 == 0
    nbt = bloc // 128          # b-tiles total
    half = bloc // 2
    nbth = half // 128         # b-tiles per half

    nc = bacc.Bacc("TRN2")
    ent = nc.dram_tensor("ent", [NE, D], mybir.dt.float32, kind="ExternalInput")
    rel = nc.dram_tensor("rel", [NRE, D], mybir.dt.float32, kind="ExternalInput")
    trip = nc.dram_tensor("trip", [bloc, 3], mybir.dt.int32, kind="ExternalInput")
    fcp = nc.dram_tensor("fcp", [NW, 128, F, D], FDT, kind="ExternalInput")
    bandsd = nc.dram_tensor("bands", [NW + 1, 128, F * W], FDT, kind="ExternalInput")
    cbias = nc.dram_tensor("cbias", [128, F], mybir.dt.float32, kind="ExternalInput")
    # scores laid out [partition, b-tile]; host transposes to batch order
    scores_d = nc.dram_tensor("scores", [128, bloc // 128], mybir.dt.float32,
                              kind="ExternalOutput")

    with tile.TileContext(nc) as tc:
        with tc.tile_pool(name="const", bufs=1) as cp, \
             tc.tile_pool(name="gath", bufs=2) as gp:
            ident = cp.tile([128, 128], mybir.dt.float32)
            make_identity(nc, ident[:])
            cb_sb = cp.tile([128, F], mybir.dt.float32)
            nc.sync.dma_start(out=cb_sb[:], in_=cbias[:, :])
            score_sb = cp.tile([128, nbt], mybir.dt.float32)

            # resident fp16 window tiles (conv positions x all local batch)
            win = {}
            for c in range(NW):
                if c == _SPLIT_C:
                    win["4a"] = cp.tile([_SPLIT_A, bloc], FDT, tag="win4a", name="win4a")
                    win["4b"] = cp.tile([_SPLIT_B, bloc], FDT, tag="win4b", name="win4b")
                else:
                    win[c] = cp.tile([K_C[c], bloc], FDT, tag=f"win{c}", name=f"win{c}")
            obj_sb = [cp.tile([128, D], mybir.dt.float32, tag=f"obj{t}", name=f"obj{t}")
                      for t in range(nbt)]

            # ---- phase 0: gathers + transposes ----
            with tc.tile_pool(name="tpsum", bufs=4, space="PSUM") as tps_pool:
                # dummy PE op so the first real transpose carries one wait
                warm = tps_pool.tile([128, 128], mybir.dt.float32, space="PSUM",
                                     bufs=1)
                nc.tensor.transpose(out=warm[:, :], in_=ident[:], identity=ident[:])
                for t in range(nbt):
                    tr = gp.tile([128, 3], mybir.dt.int32, tag="tr")
                    nc.sync.dma_start(out=tr[:], in_=trip[128 * t:128 * (t + 1), :])
                    gs = gp.tile([128, D], mybir.dt.float32, tag="gs")
                    gr = gp.tile([128, D], mybir.dt.float32, tag="gr")
                    nc.gpsimd.indirect_dma_start(
                        out=gs[:], out_offset=None, in_=ent[:, :],
                        in_offset=bass.IndirectOffsetOnAxis(ap=tr[:, 0:1], axis=0))
                    nc.gpsimd.indirect_dma_start(
                        out=gr[:], out_offset=None, in_=rel[:, :],
                        in_offset=bass.IndirectOffsetOnAxis(ap=tr[:, 1:2], axis=0))
                    nc.gpsimd.indirect_dma_start(
                        out=obj_sb[t][:], out_offset=None, in_=ent[:, :],
                        in_offset=bass.IndirectOffsetOnAxis(ap=tr[:, 2:3], axis=0))
                    for (src, d0, ln, dst) in _PIECES:
                        g = gs if src == 0 else gr
                        pt = tps_pool.tile([128, 128], mybir.dt.float32,
                                           space="PSUM", tag="pt")
                        nc.tensor.transpose(out=pt[:ln, :128],
                                            in_=g[:, d0:d0 + ln], identity=ident[:])
                        nc.vector.tensor_copy(
                            out=win[dst][:ln, 128 * t:128 * (t + 1)],
                            in_=pt[:ln, :128])

            # ---- main: two half-batches ----
            with tc.tile_pool(name="acc", bufs=1, space="PSUM") as ap_, \
                 tc.tile_pool(name="cpsum", bufs=2, space="PSUM") as cps_pool, \
                 tc.tile_pool(name="fcbuf", bufs=2) as fcb_pool, \
                 tc.tile_pool(name="bandbuf", bufs=3) as bb_pool, \
                 tc.tile_pool(name="rcbuf", bufs=3) as rc_pool, \
                 tc.tile_pool(name="dot", bufs=2) as dot_pool:
                for h in range(2):
                    hb = h * half
                    acc = [ap_.tile([128, D], mybir.dt.float32, space="PSUM",
                                    tag=f"acc{t}", name=f"acc{t}") for t in range(nbth)]
                    for c in range(NW):
                        m, k = M_C[c], K_C[c]
                        fct = fcb_pool.tile([128, F, D], FDT, tag="fct")
                        nrows = m + 1 if c == NW - 1 else m
                        nc.sync.dma_start(out=fct[:nrows, :, :],
                                          in_=fcp[c, 0:nrows, :, :])
                        bt = bb_pool.tile([128, F * W], FDT, tag="bt")
                        nc.sync.dma_start(out=bt[:k, :], in_=bandsd[c, 0:k, :])
                        if c == _SPLIT_C:
                            bt2 = bb_pool.tile([128, F * W], FDT, tag="bt2")
                            nc.sync.dma_start(out=bt2[:_SPLIT_B, :],
                                              in_=bandsd[NW, 0:_SPLIT_B, :])
                        for f in range(F):
                            cps = cps_pool.tile([128, half], mybir.dt.float32,
                                                space="PSUM", tag="cps")
                            fs = f * W
                            if c == _SPLIT_C:
                                nc.tensor.matmul(
                                    out=cps[:m, :half],
                                    lhsT=bt[:_SPLIT_A, fs:fs + m],
                                    rhs=win["4a"][:_SPLIT_A, hb:hb + half],
                                    start=True, stop=False)
                                nc.tensor.matmul(
                                    out=cps[:m, :half],
                                    lhsT=bt2[:_SPLIT_B, fs:fs + m],
                                    rhs=win["4b"][:_SPLIT_B, hb:hb + half],
                                    start=False, stop=True)
                            else:
                                nc.tensor.matmul(
                                    out=cps[:m, :half],
                                    lhsT=bt[:k, fs:fs + m],
                                    rhs=win[c][:k, hb:hb + half],
                                    start=True, stop=True)
                            rc = rc_pool.tile([128, half], FDT, tag="rc")
                            keff = m
                            if c == NW - 1 and f == F - 1:
                                # rows 96:110 get overwritten by the relu copy
                                # below, leaving row m=110 at 1.0 — it pairs
                                # with the fc_bias row packed into fcp.
                                nc.vector.memset(rc[96:128, :], 1.0)
                                keff = m + 1
                            nc.scalar.activation(
                                out=rc[:m, :], in_=cps[:m, :half],
                                func=mybir.ActivationFunctionType.Relu,
                                bias=cb_sb[:m, f:f + 1])
                            first = (c == 0 and f == 0)
                            last = (c == NW - 1 and f == F - 1)
                            for t in range(nbth):
                                nc.tensor.matmul(
                                    out=acc[t][:, :],
                                    lhsT=rc[:keff, 128 * t:128 * (t + 1)],
                                    rhs=fct[:keff, f, :],
                                    start=first, stop=last)
                    # obj dot for this half
                    for t in range(nbth):
                        g = h * nbth + t
                        prod = dot_pool.tile([128, D], mybir.dt.float32, tag="prod")
                        nc.vector.tensor_tensor(out=prod[:], in0=acc[t][:, :],
                                                in1=obj_sb[g][:],
                                                op=mybir.AluOpType.mult)
                        nc.vector.tensor_reduce(out=score_sb[:, g:g + 1],
                                                in_=prod[:],
                                                axis=mybir.AxisListType.X,
                                                op=mybir.AluOpType.add)
            nc.sync.dma_start(out=scores_d[:, :], in_=score_sb[:, :nbt])
    nc.compile()
    return nc


def _run(inputs, bloc=None, n_cores=NCORES, trace=False):
    entity_emb = np.ascontiguousarray(np.asarray(inputs["entity_emb"], np.float32))
    rel_emb = np.ascontiguousarray(np.asarray(inputs["rel_emb"], np.float32))
    triples = np.asarray(inputs["triples"]).astype(np.int32)
    bands16, fcp16, cbias = host_pack(inputs["conv_weight"], inputs["conv_bias"],
                                      inputs["fc"], inputs["fc_bias"])
    n = triples.shape[0]
    if bloc is None:
        bloc = n // n_cores
    assert n == bloc * n_cores

    nc = build_bass(bloc)
    in_maps = []
    for cid in range(n_cores):
        in_maps.append({
            "ent": entity_emb,
            "rel": rel_emb,
            "trip": np.ascontiguousarray(triples[cid * bloc:(cid + 1) * bloc]),
            "fcp": fcp16,
            "bands": bands16,
            "cbias": cbias,
        })
    res = run_bass_kernel_spmd(nc, in_maps, core_ids=list(range(n_cores)),
                               trace=trace)
    scores = np.concatenate([np.asarray(r["scores"]).T.reshape(-1)
                             for r in res.results])
    return scores.astype(np.float32), res


def kernel(**inputs) -> np.ndarray:
    scores, _ = _run(inputs)
    return scores


# revision 9
# speedup vs baseline: 1.2300x; 1.2300x over previous
"""ConvTransE decoder scoring kernel for Trainium2 (8 NeuronCores, Bass/Tile).

Math (per batch row b):
    subj = entity_emb[triples[b,0]]; rel = rel_emb[triples[b,1]]
    obj  = entity_emb[triples[b,2]]
    combined = concat(subj, rel)                      # (1024,)
    conv[f,i] = relu(sum_k combined[i+k] * w[f,k] + cb[f])   # i in [0,1022)
    proj = conv.reshape(-1) @ fc + fc_bias            # (512,)
    score = proj . obj

Strategy: data-parallel over the batch (1024 triples per core).  Per core:
  - indirect-DMA gathers of subj/rel/obj rows (all issued upfront)
  - PE transposes build "combined^T" window tiles (conv positions on the
    partition axis, batch on the free axis); windows are transposed lazily,
    two windows ahead of their first use, so they hide under main compute
  - conv as banded fp16 matmuls (host-precomputed band matrices)
  - relu on the scalar engine during PSUM->SBUF copy (conv bias folded in)
  - the 32704x512 FC contraction as fp16 matmuls accumulating in PSUM,
    streaming host-repacked fp16 fc chunks from DRAM
  - fc_bias folded in as one extra contraction row (paired with a 1.0 row)
  - final dot with obj rows on the vector engine
Batch runs in two halves of 512 so PSUM holds 4 accumulator banks + conv +
transpose banks.  The conv for chunk g+1 is emitted before the FC matmuls
of chunk g (software pipeline) so the tensor engine never waits on the
scalar-engine relu.

All matmul operand APs are kept at partition base 0 (nonzero-base operands
produce broken NEFFs on this toolchain), and vector-engine partition bases
are 32-aligned.
"""
import numpy as np

import concourse.bass as bass
import concourse.mybir as mybir
import concourse.tile as tile
from concourse import bacc
from concourse.masks import make_identity
from concourse.bass_utils import run_bass_kernel_spmd

# problem constants
NE, NRE, D, F, KS, B = 100000, 500, 512, 32, 3, 8192
NCORES = 8
CONV_LEN = 2 * D - KS + 1          # 1022
W = 114                            # conv output rows per window
NW = 9                             # number of windows (9*114 >= 1022)
M_C = [W] * (NW - 1) + [CONV_LEN - W * (NW - 1)]       # [114]*8 + [110]
K_C = [m + KS - 1 for m in M_C]                        # [116]*8 + [112]
FDT = mybir.dt.float16

# transpose source pieces per window: (source, d0, length, dest)
# source: 0=subj, 1=rel;  dest: window index, or "4a"/"4b" for the split window
_PIECES = []
for _c in range(NW):
    p0, p1 = W * _c, W * _c + K_C[_c]
    if p1 <= D:
        _PIECES.append((0, p0, p1 - p0, _c))
    elif p0 >= D:
        _PIECES.append((1, p0 - D, p1 - p0, _c))
    else:
        _PIECES.append((0, p0, D - p0, "4a"))
        _PIECES.append((1, 0, p1 - D, "4b"))
_SPLIT_C = 4
_SPLIT_A = D - W * _SPLIT_C          # 56 rows from subj
_SPLIT_B = K_C[_SPLIT_C] - _SPLIT_A  # 60 rows from rel


def host_pack(conv_weight, conv_bias, fc, fc_bias):
    """Precompute fp16 band matrices and the repacked fp16 fc chunks."""
    w = np.asarray(conv_weight, np.float32).reshape(F, KS)
    # band slabs: bands[c][p, f*W + i] = w[f, p - i]; slab 9 = rows 56.. of c=4
    bands = np.zeros((NW + 1, 128, F * W), np.float32)
    for c in range(NW):
        m, k = M_C[c], K_C[c]
        for f in range(F):
            bf = np.zeros((k, W))
            for kk in range(KS):
                idx = np.arange(m)
                bf[idx + kk, idx] = w[f, kk]
            if c == _SPLIT_C:
                bands[c, :_SPLIT_A, f * W:f * W + W] = bf[:_SPLIT_A]
                bands[NW, :_SPLIT_B, f * W:f * W + W] = bf[_SPLIT_A:]
            else:
                bands[c, :k, f * W:f * W + W] = bf
    bands16 = bands.astype(np.float16)

    fc32 = np.asarray(fc, np.float32)
    fcp = np.zeros((NW, 128, F, D), np.float32)
    for c in range(NW):
        m = M_C[c]
        rows = np.arange(m)
        for f in range(F):
            fcp[c, :m, f, :] = fc32[f * CONV_LEN + W * c + rows, :]
    # fc_bias as one extra contraction row on the last chunk (paired with a
    # 1.0 row in the relu output)
    fcp[NW - 1, M_C[NW - 1], F - 1, :] = np.asarray(fc_bias, np.float32)
    fcp16 = fcp.astype(np.float16)

    cbias = np.tile(np.asarray(conv_bias, np.float32).reshape(1, F), (128, 1))
    return bands16, fcp16, cbias.astype(np.float32)


def build_bass(bloc):
    """Build the per-core Bass module for a local batch of `bloc` triples."""
    assert bloc % 256 == 0
    nbt = bloc // 128          # b-tiles total
    half = bloc // 2
    nbth = half // 128         # b-tiles per half

    nc = bacc.Bacc("TRN2")
    ent = nc.dram_tensor("ent", [NE, D], mybir.dt.float32, kind="ExternalInput")
    rel = nc.dram_tensor("rel", [NRE, D], mybir.dt.float32, kind="ExternalInput")
    trip = nc.dram_tensor("trip", [bloc, 3], mybir.dt.int32, kind="ExternalInput")
    fcp = nc.dram_tensor("fcp", [NW, 128, F, D], FDT, kind="ExternalInput")
    bandsd = nc.dram_tensor("bands", [NW + 1, 128, F * W], FDT, kind="ExternalInput")
    cbias = nc.dram_tensor("cbias", [128, F], mybir.dt.float32, kind="ExternalInput")
    # scores laid out [partition, b-tile]; host transposes to batch order
    scores_d = nc.dram_tensor("scores", [128, bloc // 128], mybir.dt.float32,
                              kind="ExternalOutput")

    with tile.TileContext(nc) as tc:
        with tc.tile_pool(name="const", bufs=1) as cp, \
             tc.tile_pool(name="gath", bufs=2) as gp:
            ident = cp.tile([128, 128], mybir.dt.float32)
            make_identity(nc, ident[:])
            cb_sb = cp.tile([128, F], mybir.dt.float32)
            nc.sync.dma_start(out=cb_sb[:], in_=cbias[:, :])
            score_sb = cp.tile([128, nbt], mybir.dt.float32)

            # resident fp16 window tiles (conv positions x all local batch)
            win = {}
            for c in range(NW):
                if c == _SPLIT_C:
                    win["4a"] = cp.tile([_SPLIT_A, bloc], FDT, name="win4a")
                    win["4b"] = cp.tile([_SPLIT_B, bloc], FDT, name="win4b")
                else:
                    win[c] = cp.tile([K_C[c], bloc], FDT, name=f"win{c}")
            obj_sb = [cp.tile([128, D], mybir.dt.float32, name=f"obj{t}")
                      for t in range(nbt)]
            gs_sb = [cp.tile([128, D], mybir.dt.float32, name=f"gs{t}")
                     for t in range(nbt)]
            gr_sb = [cp.tile([128, D], mybir.dt.float32, name=f"gr{t}")
                     for t in range(nbt)]

            # window -> transpose pieces
            win_pieces = {}
            for (src, d0, ln, dst) in _PIECES:
                c = _SPLIT_C if dst in ("4a", "4b") else dst
                win_pieces.setdefault(c, []).append((src, d0, ln, dst))

            with tc.tile_pool(name="acc", bufs=1, space="PSUM") as ap_, \
                 tc.tile_pool(name="cpsum", bufs=2, space="PSUM") as cps_pool, \
                 tc.tile_pool(name="tpsum", bufs=2, space="PSUM") as tps_pool, \
                 tc.tile_pool(name="fcbuf", bufs=2) as fcb_pool, \
                 tc.tile_pool(name="bandbuf", bufs=3) as bb_pool, \
                 tc.tile_pool(name="rcbuf", bufs=3) as rc_pool, \
                 tc.tile_pool(name="dot", bufs=2) as dot_pool:

                # all gathers upfront: subj first (earliest consumer), then
                # rel, then obj (needed only at the end of each half)
                trs = []
                for t in range(nbt):
                    tr = gp.tile([128, 3], mybir.dt.int32, tag=f"tr{t}",
                                 name=f"tr{t}")
                    nc.sync.dma_start(out=tr[:],
                                      in_=trip[128 * t:128 * (t + 1), :])
                    trs.append(tr)
                for t in range(nbt):
                    nc.gpsimd.indirect_dma_start(
                        out=gs_sb[t][:], out_offset=None, in_=ent[:, :],
                        in_offset=bass.IndirectOffsetOnAxis(ap=trs[t][:, 0:1],
                                                            axis=0))
                for t in range(nbt):
                    nc.gpsimd.indirect_dma_start(
                        out=gr_sb[t][:], out_offset=None, in_=rel[:, :],
                        in_offset=bass.IndirectOffsetOnAxis(ap=trs[t][:, 1:2],
                                                            axis=0))
                for t in range(nbt):
                    nc.gpsimd.indirect_dma_start(
                        out=obj_sb[t][:], out_offset=None, in_=ent[:, :],
                        in_offset=bass.IndirectOffsetOnAxis(ap=trs[t][:, 2:3],
                                                            axis=0))

                def emit_window(c):
                    for (src, d0, ln, dst) in win_pieces[c]:
                        for t in range(nbt):
                            g = gs_sb[t] if src == 0 else gr_sb[t]
                            pt = tps_pool.tile([128, 128], mybir.dt.float32,
                                               space="PSUM", tag="pt",
                                               name="pt")
                            nc.tensor.transpose(out=pt[:ln, :128],
                                                in_=g[:, d0:d0 + ln],
                                                identity=ident[:])
                            nc.vector.tensor_copy(
                                out=win[dst][:ln, 128 * t:128 * (t + 1)],
                                in_=pt[:ln, :128])

                emit_window(0)
                emit_window(1)

                def emit_conv(c, f, hb, bands_c):
                    m, k = M_C[c], K_C[c]
                    bt, bt2 = bands_c
                    cps = cps_pool.tile([128, half], mybir.dt.float32,
                                        space="PSUM", tag="cps", name="cps")
                    fs = f * W
                    if c == _SPLIT_C:
                        nc.tensor.matmul(out=cps[:m, :half],
                                         lhsT=bt[:_SPLIT_A, fs:fs + m],
                                         rhs=win["4a"][:_SPLIT_A, hb:hb + half],
                                         start=True, stop=False)
                        nc.tensor.matmul(out=cps[:m, :half],
                                         lhsT=bt2[:_SPLIT_B, fs:fs + m],
                                         rhs=win["4b"][:_SPLIT_B, hb:hb + half],
                                         start=False, stop=True)
                    else:
                        nc.tensor.matmul(out=cps[:m, :half],
                                         lhsT=bt[:k, fs:fs + m],
                                         rhs=win[c][:k, hb:hb + half],
                                         start=True, stop=True)
                    rc = rc_pool.tile([128, half], FDT, tag="rc", name="rc")
                    if c == NW - 1 and f == F - 1:
                        # rows 96:m get overwritten by the relu copy below,
                        # leaving row m at 1.0 — pairs with the fc_bias row
                        # packed into fcp
                        nc.vector.memset(rc[96:128, :], 1.0)
                    nc.scalar.activation(out=rc[:m, :], in_=cps[:m, :half],
                                         func=mybir.ActivationFunctionType.Relu,
                                         bias=cb_sb[:m, f:f + 1])
                    return rc

                def emit_main(acc, state, stop):
                    rc, keff, f, fct, first = state
                    for t in range(len(acc)):
                        nc.tensor.matmul(out=acc[t][:, :],
                                         lhsT=rc[:keff, 128 * t:128 * (t + 1)],
                                         rhs=fct[:keff, f, :],
                                         start=first, stop=stop)

                for h in range(2):
                    hb = h * half
                    acc = [ap_.tile([128, D], mybir.dt.float32, space="PSUM",
                                    tag=f"acc{t}", name=f"acc{t}")
                           for t in range(nbth)]
                    prev = None
                    for c in range(NW):
                        m = M_C[c]
                        fct = fcb_pool.tile([128, F, D], FDT, tag="fct",
                                            name="fct")
                        nrows = m + 1 if c == NW - 1 else m
                        nc.sync.dma_start(out=fct[:nrows, :, :],
                                          in_=fcp[c, 0:nrows, :, :])
                        bt = bb_pool.tile([128, F * W], FDT, tag="bt", name="bt")
                        nc.sync.dma_start(out=bt[:K_C[c], :],
                                          in_=bandsd[c, 0:K_C[c], :])
                        if c == _SPLIT_C:
                            bt2 = bb_pool.tile([128, F * W], FDT, tag="bt2",
                                               name="bt2")
                            nc.sync.dma_start(out=bt2[:_SPLIT_B, :],
                                              in_=bandsd[NW, 0:_SPLIT_B, :])
                            bands_c = (bt, bt2)
                        else:
                            bands_c = (bt, None)
                        if h == 0 and c + 2 < NW:
                            emit_window(c + 2)
                        for f in range(F):
                            rc = emit_conv(c, f, hb, bands_c)
                            keff = m + 1 if (c == NW - 1 and f == F - 1) else m
                            if prev is not None:
                                emit_main(acc, prev, stop=False)
                            prev = (rc, keff, f, fct, c == 0 and f == 0)
                    emit_main(acc, prev, stop=True)
                    # obj dot for this half
                    for t in range(nbth):
                        g = h * nbth + t
                        prod = dot_pool.tile([128, D], mybir.dt.float32,
                                             tag="prod", name="prod")
                        nc.vector.tensor_tensor(out=prod[:], in0=acc[t][:, :],
                                                in1=obj_sb[g][:],
                                                op=mybir.AluOpType.mult)
                        nc.vector.tensor_reduce(out=score_sb[:, g:g + 1],
                                                in_=prod[:],
                                                axis=mybir.AxisListType.X,
                                                op=mybir.AluOpType.add)
            nc.sync.dma_start(out=scores_d[:, :], in_=score_sb[:, :nbt])
    nc.compile()
    return nc


def _run(inputs, bloc=None, n_cores=NCORES, trace=False):
    entity_emb = np.ascontiguousarray(np.asarray(inputs["entity_emb"], np.float32))
    rel_emb = np.ascontiguousarray(np.asarray(inputs["rel_emb"], np.float32))
    triples = np.asarray(inputs["triples"]).astype(np.int32)
    bands16, fcp16, cbias = host_pack(inputs["conv_weight"], inputs["conv_bias"],
                                      inputs["fc"], inputs["fc_bias"])
    n = triples.shape[0]
    if bloc is None:
        bloc = n // n_cores
    assert n == bloc * n_cores

    nc = build_bass(bloc)
    in_maps = []
    for cid in range(n_cores):
        in_maps.append({
            "ent": entity_emb,
            "rel": rel_emb,
            "trip": np.ascontiguousarray(triples[cid * bloc:(cid + 1) * bloc]),
            "fcp": fcp16,
            "bands": bands16,
            "cbias": cbias,
        })
    res = run_bass_kernel_spmd(nc, in_maps, core_ids=list(range(n_cores)),
                               trace=trace)
    scores = np.concatenate([np.asarray(r["scores"]).T.reshape(-1)
                             for r in res.results])
    return scores.astype(np.float32), res


def kernel(**inputs) -> np.ndarray:
    scores, _ = _run(inputs)
    return scores


# revision 10
# speedup vs baseline: 1.3005x; 1.0573x over previous
"""ConvTransE decoder scoring kernel for Trainium2 (8 NeuronCores, Bass/Tile).

Math (per batch row b):
    subj = entity_emb[triples[b,0]]; rel = rel_emb[triples[b,1]]
    obj  = entity_emb[triples[b,2]]
    combined = concat(subj, rel)                      # (1024,)
    conv[f,i] = relu(sum_k combined[i+k] * w[f,k] + cb[f])   # i in [0,1022)
    proj = conv.reshape(-1) @ fc + fc_bias            # (512,)
    score = proj . obj

Strategy: data-parallel over the batch (1024 triples per core).  Per core:
  - indirect-DMA gathers of subj/rel/obj rows (all issued upfront)
  - PE transposes build "combined^T" window tiles (conv positions on the
    partition axis, batch on the free axis); windows are transposed lazily,
    two windows ahead of their first use, so they hide under main compute
  - conv as banded fp16 matmuls (host-precomputed band matrices)
  - relu on the scalar engine during PSUM->SBUF copy (conv bias folded in)
  - the 32704x512 FC contraction as fp16 matmuls accumulating in PSUM,
    streaming host-repacked fp16 fc chunks from DRAM
  - fc_bias folded in as one extra contraction row (paired with a 1.0 row)
  - final dot with obj rows on the vector engine
Batch runs in two halves of 512 so PSUM holds 4 accumulator banks + conv +
transpose banks.  The conv for chunk g+1 is emitted before the FC matmuls
of chunk g (software pipeline) so the tensor engine never waits on the
scalar-engine relu.

All matmul operand APs are kept at partition base 0 (nonzero-base operands
produce broken NEFFs on this toolchain), and vector-engine partition bases
are 32-aligned.
"""
import numpy as np

import concourse.bass as bass
import concourse.mybir as mybir
import concourse.tile as tile
from concourse import bacc
from concourse.masks import make_identity
from concourse.bass_utils import run_bass_kernel_spmd

# problem constants
NE, NRE, D, F, KS, B = 100000, 500, 512, 32, 3, 8192
NCORES = 8
CONV_LEN = 2 * D - KS + 1          # 1022
W = 114                            # conv output rows per window
NW = 9                             # number of windows (9*114 >= 1022)
M_C = [W] * (NW - 1) + [CONV_LEN - W * (NW - 1)]       # [114]*8 + [110]
K_C = [m + KS - 1 for m in M_C]                        # [116]*8 + [112]
FDT = mybir.dt.float16

# transpose source pieces per window: (source, d0, length, dest)
# source: 0=subj, 1=rel;  dest: window index, or "4a"/"4b" for the split window
_PIECES = []
for _c in range(NW):
    p0, p1 = W * _c, W * _c + K_C[_c]
    if p1 <= D:
        _PIECES.append((0, p0, p1 - p0, _c))
    elif p0 >= D:
        _PIECES.append((1, p0 - D, p1 - p0, _c))
    else:
        _PIECES.append((0, p0, D - p0, "4a"))
        _PIECES.append((1, 0, p1 - D, "4b"))
_SPLIT_C = 4
_SPLIT_A = D - W * _SPLIT_C          # 56 rows from subj
_SPLIT_B = K_C[_SPLIT_C] - _SPLIT_A  # 60 rows from rel


def host_pack(conv_weight, conv_bias, fc, fc_bias):
    """Precompute fp16 band matrices and the repacked fp16 fc chunks."""
    w = np.asarray(conv_weight, np.float32).reshape(F, KS)
    # band slabs: bands[c][p, f*W + i] = w[f, p - i]; slab 9 = rows 56.. of c=4
    bands = np.zeros((NW + 1, 128, F * W), np.float32)
    for c in range(NW):
        m, k = M_C[c], K_C[c]
        for f in range(F):
            bf = np.zeros((k, W))
            for kk in range(KS):
                idx = np.arange(m)
                bf[idx + kk, idx] = w[f, kk]
            if c == _SPLIT_C:
                bands[c, :_SPLIT_A, f * W:f * W + W] = bf[:_SPLIT_A]
                bands[NW, :_SPLIT_B, f * W:f * W + W] = bf[_SPLIT_A:]
            else:
                bands[c, :k, f * W:f * W + W] = bf
    bands16 = bands.astype(np.float16)

    fc32 = np.asarray(fc, np.float32)
    fcp = np.zeros((NW, 128, F, D), np.float32)
    for c in range(NW):
        m = M_C[c]
        rows = np.arange(m)
        for f in range(F):
            fcp[c, :m, f, :] = fc32[f * CONV_LEN + W * c + rows, :]
    # fc_bias as one extra contraction row on the last chunk (paired with a
    # 1.0 row in the relu output)
    fcp[NW - 1, M_C[NW - 1], F - 1, :] = np.asarray(fc_bias, np.float32)
    fcp16 = fcp.astype(np.float16)

    cbias = np.tile(np.asarray(conv_bias, np.float32).reshape(1, F), (128, 1))
    return bands16, fcp16, cbias.astype(np.float32)


def build_bass(bloc):
    """Build the per-core Bass module for a local batch of `bloc` triples."""
    assert bloc % 256 == 0
    nbt = bloc // 128          # b-tiles total
    half = bloc // 2
    nbth = half // 128         # b-tiles per half

    nc = bacc.Bacc("TRN2")
    ent = nc.dram_tensor("ent", [NE, D], mybir.dt.float32, kind="ExternalInput")
    rel = nc.dram_tensor("rel", [NRE, D], mybir.dt.float32, kind="ExternalInput")
    trip = nc.dram_tensor("trip", [bloc, 3], mybir.dt.int32, kind="ExternalInput")
    fcp = nc.dram_tensor("fcp", [NW, 128, F, D], FDT, kind="ExternalInput")
    bandsd = nc.dram_tensor("bands", [NW + 1, 128, F * W], FDT, kind="ExternalInput")
    cbias = nc.dram_tensor("cbias", [128, F], mybir.dt.float32, kind="ExternalInput")
    # scores laid out [partition, b-tile]; host transposes to batch order
    scores_d = nc.dram_tensor("scores", [128, bloc // 128], mybir.dt.float32,
                              kind="ExternalOutput")

    with tile.TileContext(nc) as tc:
        with tc.tile_pool(name="const", bufs=1) as cp, \
             tc.tile_pool(name="gath", bufs=2) as gp:
            ident = cp.tile([128, 128], mybir.dt.float32)
            make_identity(nc, ident[:])
            cb_sb = cp.tile([128, F], mybir.dt.float32)
            nc.sync.dma_start(out=cb_sb[:], in_=cbias[:, :])
            score_sb = cp.tile([128, nbt], mybir.dt.float32)

            # resident fp16 window tiles (conv positions x all local batch)
            win = {}
            for c in range(NW):
                if c == _SPLIT_C:
                    win["4a"] = cp.tile([_SPLIT_A, bloc], FDT, name="win4a")
                    win["4b"] = cp.tile([_SPLIT_B, bloc], FDT, name="win4b")
                else:
                    win[c] = cp.tile([K_C[c], bloc], FDT, name=f"win{c}")
            obj_sb = [cp.tile([128, D], mybir.dt.float32, name=f"obj{t}")
                      for t in range(nbt)]
            gs_sb = [cp.tile([128, D], mybir.dt.float32, name=f"gs{t}")
                     for t in range(nbt)]
            gr_sb = [cp.tile([128, D], mybir.dt.float32, name=f"gr{t}")
                     for t in range(nbt)]

            # window -> transpose pieces
            win_pieces = {}
            for (src, d0, ln, dst) in _PIECES:
                c = _SPLIT_C if dst in ("4a", "4b") else dst
                win_pieces.setdefault(c, []).append((src, d0, ln, dst))

            with tc.tile_pool(name="acc", bufs=1, space="PSUM") as ap_, \
                 tc.tile_pool(name="cpsum", bufs=2, space="PSUM") as cps_pool, \
                 tc.tile_pool(name="tpsum", bufs=2, space="PSUM") as tps_pool, \
                 tc.tile_pool(name="fcbuf", bufs=2) as fcb_pool, \
                 tc.tile_pool(name="bandbuf", bufs=3) as bb_pool, \
                 tc.tile_pool(name="rcbuf", bufs=3) as rc_pool, \
                 tc.tile_pool(name="dot", bufs=2) as dot_pool:

                # all gathers upfront: subj first (earliest consumer), then
                # rel, then obj (needed only at the end of each half)
                trs = []
                for t in range(nbt):
                    tr = gp.tile([128, 3], mybir.dt.int32, tag=f"tr{t}",
                                 name=f"tr{t}")
                    nc.sync.dma_start(out=tr[:],
                                      in_=trip[128 * t:128 * (t + 1), :])
                    trs.append(tr)
                for t in range(nbt):
                    nc.gpsimd.indirect_dma_start(
                        out=gs_sb[t][:], out_offset=None, in_=ent[:, :],
                        in_offset=bass.IndirectOffsetOnAxis(ap=trs[t][:, 0:1],
                                                            axis=0))
                for t in range(nbt):
                    nc.gpsimd.indirect_dma_start(
                        out=gr_sb[t][:], out_offset=None, in_=rel[:, :],
                        in_offset=bass.IndirectOffsetOnAxis(ap=trs[t][:, 1:2],
                                                            axis=0))
                for t in range(nbt):
                    nc.gpsimd.indirect_dma_start(
                        out=obj_sb[t][:], out_offset=None, in_=ent[:, :],
                        in_offset=bass.IndirectOffsetOnAxis(ap=trs[t][:, 2:3],
                                                            axis=0))

                def emit_window(c):
                    for (src, d0, ln, dst) in win_pieces[c]:
                        for t in range(nbt):
                            g = gs_sb[t] if src == 0 else gr_sb[t]
                            pt = tps_pool.tile([128, 128], mybir.dt.float32,
                                               space="PSUM", tag="pt",
                                               name="pt")
                            nc.tensor.transpose(out=pt[:ln, :128],
                                                in_=g[:, d0:d0 + ln],
                                                identity=ident[:])
                            nc.vector.tensor_copy(
                                out=win[dst][:ln, 128 * t:128 * (t + 1)],
                                in_=pt[:ln, :128])

                emit_window(0)
                emit_window(1)

                def dma_window(c):
                    """Prefetch one window's fc chunks (split across both
                    HWDGE rings) and band slabs."""
                    m = M_C[c]
                    nrows = m + 1 if c == NW - 1 else m
                    fa = fcb_pool.tile([128, 16, D], FDT, tag="fcta",
                                       name="fcta", bufs=2)
                    fb = fcb_pool.tile([128, 16, D], FDT, tag="fctb",
                                       name="fctb", bufs=2)
                    nc.sync.dma_start(out=fa[:nrows, :, :],
                                      in_=fcp[c, 0:nrows, 0:16, :])
                    nc.scalar.dma_start(out=fb[:nrows, :, :],
                                        in_=fcp[c, 0:nrows, 16:32, :])
                    bt = bb_pool.tile([128, F * W], FDT, tag="bt", name="bt",
                                      bufs=2)
                    nc.sync.dma_start(out=bt[:K_C[c], :],
                                      in_=bandsd[c, 0:K_C[c], :])
                    bt2 = None
                    if c == _SPLIT_C:
                        bt2 = bb_pool.tile([128, F * W], FDT, tag="bt2",
                                           name="bt2", bufs=1)
                        nc.scalar.dma_start(out=bt2[:_SPLIT_B, :],
                                            in_=bandsd[NW, 0:_SPLIT_B, :])
                    return (fa, fb, bt, bt2)

                def emit_conv(c, f, hb, tiles):
                    m, k = M_C[c], K_C[c]
                    fa, fb, bt, bt2 = tiles
                    cps = cps_pool.tile([128, half], mybir.dt.float32,
                                        space="PSUM", tag="cps", name="cps")
                    fs = f * W
                    if c == _SPLIT_C:
                        nc.tensor.matmul(out=cps[:m, :half],
                                         lhsT=bt[:_SPLIT_A, fs:fs + m],
                                         rhs=win["4a"][:_SPLIT_A, hb:hb + half],
                                         start=True, stop=False)
                        nc.tensor.matmul(out=cps[:m, :half],
                                         lhsT=bt2[:_SPLIT_B, fs:fs + m],
                                         rhs=win["4b"][:_SPLIT_B, hb:hb + half],
                                         start=False, stop=True)
                    else:
                        nc.tensor.matmul(out=cps[:m, :half],
                                         lhsT=bt[:k, fs:fs + m],
                                         rhs=win[c][:k, hb:hb + half],
                                         start=True, stop=True)
                    rc = rc_pool.tile([128, half], FDT, tag="rc", name="rc")
                    if c == NW - 1 and f == F - 1:
                        # rows 96:m get overwritten by the relu copy below,
                        # leaving row m at 1.0 — pairs with the fc_bias row
                        # packed into fcp
                        nc.vector.memset(rc[96:128, :], 1.0)
                    nc.scalar.activation(out=rc[:m, :], in_=cps[:m, :half],
                                         func=mybir.ActivationFunctionType.Relu,
                                         bias=cb_sb[:m, f:f + 1])
                    return rc

                def emit_main(acc, state, stop):
                    rc, keff, f, tiles, first = state
                    fct = tiles[0] if f < 16 else tiles[1]
                    for t in range(len(acc)):
                        nc.tensor.matmul(out=acc[t][:, :],
                                         lhsT=rc[:keff, 128 * t:128 * (t + 1)],
                                         rhs=fct[:keff, f % 16, :],
                                         start=first, stop=stop)

                seq = [(h, c) for h in range(2) for c in range(NW)]
                pref = dma_window(seq[0][1])
                acc = None
                for idx, (h, c) in enumerate(seq):
                    hb = h * half
                    m = M_C[c]
                    if c == 0:
                        acc = [ap_.tile([128, D], mybir.dt.float32,
                                        space="PSUM", tag=f"acc{t}",
                                        name=f"acc{t}") for t in range(nbth)]
                        prev = None
                    tiles = pref
                    if idx + 1 < len(seq):
                        pref = dma_window(seq[idx + 1][1])
                    if h == 0 and c + 2 < NW:
                        emit_window(c + 2)
                    for f in range(F):
                        rc = emit_conv(c, f, hb, tiles)
                        keff = m + 1 if (c == NW - 1 and f == F - 1) else m
                        if prev is not None:
                            emit_main(acc, prev, stop=False)
                        prev = (rc, keff, f, tiles, c == 0 and f == 0)
                    if c == NW - 1:
                        emit_main(acc, prev, stop=True)
                        # obj dot for this half
                        for t in range(nbth):
                            g = h * nbth + t
                            prod = dot_pool.tile([128, D], mybir.dt.float32,
                                                 tag="prod", name="prod")
                            nc.vector.tensor_tensor(out=prod[:],
                                                    in0=acc[t][:, :],
                                                    in1=obj_sb[g][:],
                                                    op=mybir.AluOpType.mult)
                            nc.vector.tensor_reduce(out=score_sb[:, g:g + 1],
                                                    in_=prod[:],
                                                    axis=mybir.AxisListType.X,
                                                    op=mybir.AluOpType.add)
            nc.sync.dma_start(out=scores_d[:, :], in_=score_sb[:, :nbt])
    nc.compile()
    return nc


def _run(inputs, bloc=None, n_cores=NCORES, trace=False):
    entity_emb = np.ascontiguousarray(np.asarray(inputs["entity_emb"], np.float32))
    rel_emb = np.ascontiguousarray(np.asarray(inputs["rel_emb"], np.float32))
    triples = np.asarray(inputs["triples"]).astype(np.int32)
    bands16, fcp16, cbias = host_pack(inputs["conv_weight"], inputs["conv_bias"],
                                      inputs["fc"], inputs["fc_bias"])
    n = triples.shape[0]
    if bloc is None:
        bloc = n // n_cores
    assert n == bloc * n_cores

    nc = build_bass(bloc)
    in_maps = []
    for cid in range(n_cores):
        in_maps.append({
            "ent": entity_emb,
            "rel": rel_emb,
            "trip": np.ascontiguousarray(triples[cid * bloc:(cid + 1) * bloc]),
            "fcp": fcp16,
            "bands": bands16,
            "cbias": cbias,
        })
    res = run_bass_kernel_spmd(nc, in_maps, core_ids=list(range(n_cores)),
                               trace=trace)
    scores = np.concatenate([np.asarray(r["scores"]).T.reshape(-1)
                             for r in res.results])
    return scores.astype(np.float32), res


def kernel(**inputs) -> np.ndarray:
    scores, _ = _run(inputs)
    return scores
